# revision 1
# baseline (speedup 1.0000x reference)
"""Coordinate-descent (alternating Gauss-Seidel) kernel for Trainium2.

B=4 factorizations x ~ u @ v^T, M=N=4096, R=32.
  u_new = GS-sweep(a1 = x@v,   b1 = v^T v, u)
  v_new = GS-sweep(a2 = x^T@u_new, b2 = u_new^T u_new, v)

8 cores; core c owns rows [c*512,(c+1)*512) of all batches (u rows, and the
same n-range of v delivered by one fused ReduceScatter of partial a2/b2).

Pipeline: per batch: phase1 (stream x: cast bf16 -> persistent SBUF,
PE-transpose, dense a1 MM burst) -> per-batch u GS sweep -> phase2 partials
+ RS-input DMAs (all overlap the next batch's phase1). One ReduceScatter,
v-transposes hoisted before it, batched v GS sweep after.
"""

import os
from contextlib import ExitStack

import numpy as np

import concourse.bass as bass
import concourse.tile as tile
from concourse import bacc, mybir
from concourse.bass import ds
from concourse.bass_utils import run_bass_kernel_spmd
from concourse.masks import make_identity

B, M, N, R = 4, 4096, 4096, 32
NCORES = 8
MS = M // NCORES          # 512 rows per core per batch
MC = MS // 128            # 4 m-chunks of 128
NG = N // 512             # 8 n-groups of 512
NCH = N // 128            # 32 n-chunks of 128
BMC = B * MC              # 16
CHUNK = B * (MS + R)      # 2176 rows per core in the fused ReduceScatter
EPS = 1e-8
FP32 = mybir.dt.float32
BF16 = mybir.dt.bfloat16
ALU = mybir.AluOpType

_CACHE = {}
LAST_RESULT = None


def _gs_sweep(nc, gsp, pmisc, ident_f, u_ap, a_ap, b_sbs, ball_ap, nb,
              unew, pre_uT=None):
    """Gauss-Seidel sweep over nb batches at once.

    u_ap/a_ap/unew: [128, nb*MC, R] fp32 APs; b_sbs: nb [R,R] grams (SBUF);
    ball_ap: [128, nb, R, R] fp32 replicated grams; pre_uT: optional
    precomputed list of [R, MC, 128] transposed-factor tiles.
    """
    nmc = nb * MC
    s = gsp.tile([128, BMC, R], FP32, tag="s", name="s")[:, :nmc, :]
    for bb in range(nb):
        if pre_uT is None:
            puT = pmisc.tile([R, MC, 128], FP32, tag="pm")
            for i in range(MC):
                nc.tensor.transpose(puT[:, i], u_ap[:, bb * MC + i, :],
                                    ident_f)
            uT = gsp.tile([R, MC, 128], FP32, tag="uT")
            nc.vector.tensor_copy(uT[:], puT[:])
        else:
            uT = pre_uT[bb]
        ps = pmisc.tile([128, MC, R], FP32, tag="pm")
        for i in range(MC):
            nc.tensor.matmul(ps[:, i], lhsT=uT[:, i], rhs=b_sbs[bb][:],
                             start=True, stop=True)
        nc.vector.tensor_copy(s[:, bb * MC:(bb + 1) * MC, :], ps[:])

    brr16 = gsp.tile([128, BMC, R], FP32, tag="brr16", name="brr16")[:, :nmc, :]
    for bb in range(nb):
        diag_bc = bass.AP(ball_ap.tensor, ball_ap.offset + bb * R * R,
                          [ball_ap.ap[0], [0, MC], [R + 1, R]])
        nc.vector.tensor_copy(brr16[:, bb * MC:(bb + 1) * MC, :], diag_bc)
    inv16 = gsp.tile([128, BMC, R], FP32, tag="inv16", name="inv16")[:, :nmc, :]
    nc.vector.tensor_scalar_add(inv16[:], brr16[:], EPS)
    nc.vector.reciprocal(inv16[:], inv16[:])

    app = gsp.tile([128, BMC, R], FP32, tag="app", name="app")[:, :nmc, :]
    nc.vector.scalar_tensor_tensor(out=app[:], in0=a_ap, scalar=EPS,
                                   in1=inv16[:], op0=ALU.add, op1=ALU.mult)

    t1 = gsp.tile([128, BMC], FP32, tag="t1", name="t1")[:, :nmc]
    delta = gsp.tile([128, BMC, 1], FP32, tag="delta")
    tmp = gsp.tile([128, BMC, R - 1], FP32, tag="tmp")
    dap = delta[:]
    tap = tmp[:]
    for r in range(R):
        nc.vector.tensor_tensor(out=t1[:], in0=u_ap[:, :, r],
                                in1=brr16[:, :, r], op=ALU.mult)
        nc.vector.tensor_tensor(out=t1[:], in0=t1[:], in1=s[:, :, r],
                                op=ALU.subtract)
        nc.vector.tensor_tensor(out=t1[:], in0=t1[:], in1=inv16[:, :, r],
                                op=ALU.mult)
        nc.vector.tensor_tensor(out=unew[:, :, r], in0=t1[:],
                                in1=app[:, :, r], op=ALU.add)
        if r < R - 1:
            tail = R - 1 - r
            nc.vector.tensor_tensor(out=delta[:, :nmc, 0],
                                    in0=unew[:, :, r], in1=u_ap[:, :, r],
                                    op=ALU.subtract)
            d_bc = bass.AP(dap.tensor, dap.offset,
                           [dap.ap[0], [MC, nb], [1, MC], [0, tail]])
            brow_bc = bass.AP(ball_ap.tensor,
                              ball_ap.offset + r * R + r + 1,
                              [ball_ap.ap[0], [R * R, nb], [0, MC],
                               [1, tail]])
            t_out = bass.AP(tap.tensor, tap.offset,
                            [tap.ap[0], [MC * (R - 1), nb], [R - 1, MC],
                             [1, tail]])
            nc.vector.tensor_tensor(out=t_out, in0=d_bc, in1=brow_bc,
                                    op=ALU.mult)
            nc.vector.tensor_tensor(out=s[:, :, r + 1:],
                                    in0=s[:, :, r + 1:],
                                    in1=tmp[:, :nmc, :tail], op=ALU.add)


def _build():
    nc = bacc.Bacc("TRN2", target_bir_lowering=False, debug=False,
                   num_devices=NCORES)

    x_my = nc.dram_tensor("x_my", [B, MS, N], FP32, kind="ExternalInput").ap()
    u_my = nc.dram_tensor("u_my", [B, MS, R], FP32, kind="ExternalInput").ap()
    v_full = nc.dram_tensor("v_full", [B, N, R], FP32,
                            kind="ExternalInput").ap()
    v_my = nc.dram_tensor("v_my", [B, MS, R], FP32, kind="ExternalInput").ap()
    u_out = nc.dram_tensor("u_out", [B, MS, R], FP32,
                           kind="ExternalOutput").ap()
    v_out = nc.dram_tensor("v_out", [B, MS, R], FP32,
                           kind="ExternalOutput").ap()

    rs_in = nc.dram_tensor("rs_in", [NCORES * CHUNK, R], FP32)
    rs_out = nc.dram_tensor("rs_out", [CHUNK, R], FP32)
    b1_scr = nc.dram_tensor("b1_scr", [B, R, R], FP32)

    with tile.TileContext(nc) as tc, ExitStack() as ctx:
        const = ctx.enter_context(tc.tile_pool(name="const", bufs=1))
        big = ctx.enter_context(tc.tile_pool(name="big", bufs=1))
        xl = ctx.enter_context(tc.tile_pool(name="xl", bufs=4))
        xt = ctx.enter_context(tc.tile_pool(name="xt", bufs=1))
        xnatp = ctx.enter_context(tc.tile_pool(name="xnatp", bufs=2))
        vpool = ctx.enter_context(tc.tile_pool(name="vp", bufs=1))
        gsp = ctx.enter_context(tc.tile_pool(name="gsp", bufs=1))
        sm = ctx.enter_context(tc.tile_pool(name="sm", bufs=2))
        ppt = ctx.enter_context(tc.tile_pool(name="ppt", bufs=3, space="PSUM"))
        pa1p = ctx.enter_context(tc.tile_pool(name="pa1", bufs=1,
                                              space="PSUM"))
        pa2p = ctx.enter_context(tc.tile_pool(name="pa2", bufs=2,
                                              space="PSUM"))
        pmisc = ctx.enter_context(tc.tile_pool(name="pmisc", bufs=2,
                                               space="PSUM"))

        ident_b = const.tile([128, 128], BF16)
        make_identity(nc, ident_b)
        ident_f = const.tile([128, 128], FP32)
        make_identity(nc, ident_f)

        unew_all = big.tile([128, BMC, R], FP32)
        ball = big.tile([128, B, R, R], FP32)
        u_all = big.tile([128, BMC, R], FP32)
        a_all = big.tile([128, BMC, R], FP32)
        un_b = big.tile([128, BMC, R], BF16)

        b1_sbs = []
        xnat_tiles = {}
        for b in range(B):
            x_nat = xnatp.tile([128, MC, N], BF16, tag="xnat", name="xnat")
            xnat_tiles[b] = x_nat
            # ---------- v load + b1 = v^T v ----------
            v32 = vpool.tile([128, NCH, R], FP32, tag="v32")
            nc.sync.dma_start(v32[:],
                              v_full[b].rearrange("(c p) r -> p c r", p=128))
            vb = vpool.tile([128, NCH, R], BF16, tag="vb")
            nc.vector.tensor_copy(vb[:], v32[:])

            pb1 = pmisc.tile([R, R], FP32, tag="pm")
            for c in range(NCH):
                nc.tensor.matmul(pb1[:], lhsT=vb[:, c], rhs=vb[:, c],
                                 start=(c == 0), stop=(c == NCH - 1))
            b1_sb = sm.tile([R, R], FP32, tag=f"b1_{b}")
            nc.vector.tensor_copy(b1_sb[:], pb1[:])
            b1_sbs.append(b1_sb)
            nc.sync.dma_start(b1_scr.ap()[b], b1_sb[:])
            src = b1_scr.ap()[b]
            nc.sync.dma_start(
                ball[:, b], bass.AP(src.tensor, src.offset,
                                    [[0, 128], [R, R], [1, R]]))

            # ---------- phase 1: stream x, transpose; then dense MM burst ---
            xT = xt.tile([128, NCH, MS], BF16, tag="xT")
            for j in range(NG):
                for i in range(MC):
                    xload = xl.tile([128, 512], FP32, tag="xload")
                    nc.sync.dma_start(
                        xload[:],
                        x_my[b, i * 128:(i + 1) * 128, j * 512:(j + 1) * 512])
                    nc.scalar.copy(
                        x_nat[:, i, j * 512:(j + 1) * 512], xload[:])
                    pt = ppt.tile([128, 4, 128], BF16, tag="pt")
                    for k in range(4):
                        nc.tensor.transpose(
                            pt[:, k],
                            x_nat[:, i,
                                  (j * 4 + k) * 128:(j * 4 + k + 1) * 128],
                            ident_b)
                    nc.scalar.copy(
                        xT[:, j * 4:(j + 1) * 4, i * 128:(i + 1) * 128],
                        pt[:])
            pa1 = pa1p.tile([R, MS], FP32, tag="pa1")
            for c in range(NCH):
                nc.tensor.matmul(pa1[:], lhsT=vb[:, c], rhs=xT[:, c],
                                 start=(c == 0), stop=(c == NCH - 1))
            a1T_sb = sm.tile([R, MS], FP32, tag="a1T")
            nc.vector.tensor_copy(a1T_sb[:], pa1[:])

            # a natural + u load
            nc.sync.dma_start(u_all[:, b * MC:(b + 1) * MC, :],
                              u_my[b].rearrange("(i p) r -> p i r", p=128))
            pA = pmisc.tile([128, MC, R], FP32, tag="pm")
            for i in range(MC):
                nc.tensor.transpose(pA[:, i],
                                    a1T_sb[:, i * 128:(i + 1) * 128],
                                    ident_f[:R, :R])
            nc.vector.tensor_copy(a_all[:, b * MC:(b + 1) * MC, :], pA[:])

            # ---------- per-batch u GS sweep (overlaps next phase1) -------
            sl = slice(b * MC, (b + 1) * MC)
            _gs_sweep(nc, gsp, pmisc, ident_f, u_all[:, sl, :],
                      a_all[:, sl, :], [b1_sb], ball[:, b:b + 1],
                      1, unew_all[:, sl, :])
            nc.sync.dma_start(u_out[b].rearrange("(i p) r -> p i r", p=128),
                              unew_all[:, sl, :])
            nc.vector.tensor_copy(un_b[:, sl, :], unew_all[:, sl, :])

            # ---------- phase 2 partials + RS-input DMAs ------------------
            for g in range(NG):
                pa2 = pa2p.tile([128, 4, R], FP32, tag="pa2")
                for k in range(4):
                    nblk = g * 4 + k
                    for i in range(MC):
                        nc.tensor.matmul(
                            pa2[:, k],
                            lhsT=xnat_tiles[b][:, i,
                                       nblk * 128:(nblk + 1) * 128],
                            rhs=un_b[:, b * MC + i], start=(i == 0),
                            stop=(i == MC - 1))
                a2st = sm.tile([128, 4, R], FP32, tag="a2st")
                nc.vector.tensor_copy(a2st[:], pa2[:])
                dst = rs_in.ap()
                nc.sync.dma_start(
                    bass.AP(dst.tensor,
                            dst.offset + (g * CHUNK + b * (MS + R)) * R,
                            [[R, 128], [128 * R, 4], [1, R]]),
                    a2st[:])

            pb2 = pmisc.tile([R, R], FP32, tag="pm")
            for i in range(MC):
                nc.tensor.matmul(pb2[:], lhsT=un_b[:, b * MC + i],
                                 rhs=un_b[:, b * MC + i], start=(i == 0),
                                 stop=(i == MC - 1))
            b2st = sm.tile([R, R], FP32, tag="b2st")
            nc.vector.tensor_copy(b2st[:], pb2[:])
            for c in range(NCORES):
                nc.sync.dma_start(
                    rs_in.ap()[ds(c * CHUNK + b * (MS + R) + MS, R), :],
                    b2st[:])

        # ---------- v loads + transposes (overlap RS) ---------------------
        v_all = big.tile([128, BMC, R], FP32)
        vT_tiles = []
        for b in range(B):
            nc.sync.dma_start(v_all[:, b * MC:(b + 1) * MC, :],
                              v_my[b].rearrange("(i p) r -> p i r", p=128))
            pvT = pmisc.tile([R, MC, 128], FP32, tag="pm")
            for i in range(MC):
                nc.tensor.transpose(pvT[:, i], v_all[:, b * MC + i, :],
                                    ident_f)
            vT = sm.tile([R, MC, 128], FP32, tag=f"vT_{b}")
            nc.vector.tensor_copy(vT[:], pvT[:])
            vT_tiles.append(vT)

        nc.gpsimd.collective_compute(
            "ReduceScatter", ALU.add, replica_groups=[list(range(NCORES))],
            ins=[rs_in.ap()], outs=[rs_out.ap()])

        # ---------- batched v GS ------------------------------------------
        a2_all = big.tile([128, BMC, R], FP32)
        b2_sbs = []
        for b in range(B):
            nc.sync.dma_start(
                a2_all[:, b * MC:(b + 1) * MC, :],
                rs_out.ap()[ds(b * (MS + R), MS), :].rearrange(
                    "(i p) r -> p i r", p=128))
            b2_sb = sm.tile([R, R], FP32, tag=f"b2_{b}")
            nc.sync.dma_start(b2_sb[:],
                              rs_out.ap()[ds(b * (MS + R) + MS, R), :])
            b2_sbs.append(b2_sb)
            src = rs_out.ap()
            nc.sync.dma_start(
                ball[:, b],
                bass.AP(src.tensor, src.offset + (b * (MS + R) + MS) * R,
                        [[0, 128], [R, R], [1, R]]))

        vnew = big.tile([128, BMC, R], FP32)
        _gs_sweep(nc, gsp, pmisc, ident_f, v_all[:], a2_all[:], b2_sbs,
                  ball[:], B, vnew[:], pre_uT=vT_tiles)
        for b in range(B):
            nc.sync.dma_start(v_out[b].rearrange("(i p) r -> p i r", p=128),
                              vnew[:, b * MC:(b + 1) * MC, :])

    nc.compile()
    return nc


def kernel(x, u, v):
    global LAST_RESULT
    if "nc" not in _CACHE:
        _CACHE["nc"] = _build()
    nc = _CACHE["nc"]

    x = np.ascontiguousarray(x, dtype=np.float32)
    u = np.ascontiguousarray(u, dtype=np.float32)
    v = np.ascontiguousarray(v, dtype=np.float32)

    in_maps = []
    for c in range(NCORES):
        sl = slice(c * MS, (c + 1) * MS)
        in_maps.append({
            "x_my": np.ascontiguousarray(x[:, sl, :]),
            "u_my": np.ascontiguousarray(u[:, sl, :]),
            "v_full": v,
            "v_my": np.ascontiguousarray(v[:, sl, :]),
        })

    res = run_bass_kernel_spmd(nc, in_maps, list(range(NCORES)),
                               trace=os.environ.get("KBENCH_TRACE") == "1")
    LAST_RESULT = res
    u_new = np.concatenate([res.results[c]["u_out"] for c in range(NCORES)],
                           axis=1)
    v_new = np.concatenate([res.results[c]["v_out"] for c in range(NCORES)],
                           axis=1)
    return (u_new, v_new)



# revision 9
# speedup vs baseline: 1.4592x; 1.4592x over previous
"""Coordinate-descent (alternating Gauss-Seidel) kernel for Trainium2, v2.

B=4 factorizations x ~ u @ v^T, M=N=4096, R=32.
The per-column GS sweep is algebraically a triangular solve:
    u_new = (a + eps - u @ B_sl) @ M^{-1},   M = diag(B)+eps + triu(B,1)
with B = v^T v.  M^{-1} is applied exactly via the nilpotent factorization
    (I+W)^{-1} = (I-W)(I+W^2)(I+W^4)(I+W^8)(I+W^16),  W = triu(B,1) D'^{-1}
so the whole half-step is a handful of PE matmuls instead of a 32-step
vector-engine recurrence.  All work stays in transposed [R, m] space; only
small [128,32] tiles are PE-transposed at the edges.

8 cores; core c owns rows [c*512,(c+1)*512) of all batches.  Phase 2
partials (a2T = u_new^T x, b2 = u_new^T u_new) go through one fused
per-batch ReduceScatter that overlaps the next batch's phase 1.
"""

import os
from contextlib import ExitStack

import numpy as np

import concourse.bass as bass
import concourse.tile as tile
from concourse import bacc, mybir
from concourse.bass import ds
from concourse.bass_utils import run_bass_kernel_spmd
from concourse.masks import make_identity, make_lower_triangular

B, M, N, R = 4, 4096, 4096, 32
NCORES = 8
MS = M // NCORES          # 512 rows per core per batch
MC = MS // 128            # 4 m-chunks of 128
NG = N // 512             # 8 n-groups of 512
NCH = N // 128            # 32 n-chunks of 128
EPS = 1e-8
F32 = mybir.dt.float32
F32R = mybir.dt.float32r
BF16 = mybir.dt.bfloat16
ALU = mybir.AluOpType
AX = mybir.AxisListType

_CACHE = {}
LAST_RESULT = None


def _solve(nc, smp, zsb, pwp, punp, zps, consts, b_sb, at_sb, xT_sb, out32,
           outb16, tg):
    """Closed-form GS half-step in transposed space.

    b_sb: [R,R] f32r Gram (SBUF).  at_sb: [R,512] f32r = a^T + eps.
    xT_sb: [R,512] f32r old-factor transpose.  zps(tag) -> psum [R,512] tile.
    Writes natural-layout result into out32 (f32) and outb16 (bf16 or None).
    """
    ident32_r, masksl_r, eye_r = consts

    # --- diagonal, inv, strict-lower, W = V^T ---
    bd = smp.tile([R, R], F32R, tag=f"bd{tg}", name="bd")
    nc.vector.tensor_tensor(out=bd[:], in0=b_sb[:], in1=eye_r, op=ALU.mult)
    d_p = smp.tile([R, 1], F32, tag=f"dp{tg}", name="d_p")
    inv_p = smp.tile([R, 1], F32, tag=f"ip{tg}", name="inv_p")
    nc.vector.tensor_reduce(d_p[:], bd[:], axis=AX.X, op=ALU.add)
    nc.vector.tensor_scalar_add(inv_p[:], d_p[:], EPS)
    nc.vector.reciprocal(inv_p[:], inv_p[:])
    invb = bass.AP(inv_p[:].tensor, inv_p[:].offset, [inv_p[:].ap[0], [0, R]])

    bsl = smp.tile([R, R], F32R, tag=f"bsl{tg}", name="bsl")
    nc.vector.tensor_tensor(out=bsl[:], in0=b_sb[:], in1=masksl_r,
                            op=ALU.mult)
    vw = smp.tile([R, R], F32R, tag=f"vw{tg}", name="vw")  # V = W^T
    nc.vector.tensor_tensor(out=vw[:], in0=bsl[:], in1=invb, op=ALU.mult)

    # vw transpose output must be f32r; borrow a [32,32] sub-AP of the
    # f32r pun slot (pw MM tiles below must be f32)
    pwt = punp.tile([128, MC, R], F32R, tag="pun", name="pwt")
    nc.tensor.transpose(pwt[:R, 0, :], vw[:], ident32_r)
    w1 = smp.tile([R, R], F32R, tag=f"w1{tg}", name="w1")
    nc.scalar.copy(w1[:], pwt[:R, 0, :])

    # --- squarings: keep (Wk, WkT) pairs; W16T not needed ---
    def _mm_small(lhsT, rhs, tagn):
        p = pwp.tile([R, R], F32, tag="pw", name="pmm")
        nc.tensor.matmul(p[:], lhsT=lhsT[:], rhs=rhs[:], start=True,
                         stop=True)
        s = smp.tile([R, R], F32R, tag=f"{tagn}{tg}", name=tagn)
        nc.scalar.copy(s[:], p[:])
        return s

    w2 = _mm_small(vw, w1, "w2")     # W^T.T @ W = W@W
    w2t = _mm_small(w1, vw, "w2t")   # W.T @ W^T = (W@W)^T
    w4 = _mm_small(w2t, w2, "w4")
    w4t = _mm_small(w2, w2t, "w4t")
    w8 = _mm_small(w4t, w4, "w8")
    w8t = _mm_small(w4, w4t, "w8t")
    w16 = _mm_small(w8t, w8, "w16")

    # --- sT = B_sl^T @ xT ; z0 = inv_p * (at - sT) ---
    ps = zps("s")
    nc.tensor.matmul(ps[:], lhsT=bsl[:], rhs=xT_sb[:], start=True, stop=True)
    z = zsb.tile([R, MS], F32R, tag=f"z{tg}", name="z0")
    nc.vector.scalar_tensor_tensor(out=z[:], in0=ps[:], scalar=-1.0,
                                   in1=at_sb[:], op0=ALU.mult, op1=ALU.add)
    nc.vector.tensor_scalar(out=z[:], in0=z[:], scalar1=inv_p[:],
                            scalar2=None, op0=ALU.mult)

    # --- z <- z + sign * Wk^T @ z  (exact inverse chain) ---
    for wk, sign in ((w1, -1.0), (w2, 1.0), (w4, 1.0), (w8, 1.0),
                     (w16, 1.0)):
        pz = zps("z")
        nc.tensor.matmul(pz[:], lhsT=wk[:], rhs=z[:], start=True, stop=True)
        zn = zsb.tile([R, MS], F32R, tag=f"z{tg}", name="zn")
        nc.vector.scalar_tensor_tensor(out=zn[:], in0=pz[:], scalar=sign,
                                       in1=z[:], op0=ALU.mult, op1=ALU.add)
        z = zn

    # --- back to natural [128, MC, R] ---
    pun = punp.tile([128, MC, R], F32R, tag="pun", name="pun")
    for i in range(MC):
        nc.tensor.transpose(pun[:, i], z[:, i * 128:(i + 1) * 128],
                            ident32_r)
    nc.scalar.copy(out32[:], pun[:])
    if outb16 is not None:
        nc.vector.tensor_copy(outb16[:], pun[:])


def _build():
    nc = bacc.Bacc("TRN2", target_bir_lowering=False, debug=False,
                   num_devices=NCORES)

    x_my = nc.dram_tensor("x_my", [B, MS, N], F32, kind="ExternalInput").ap()
    u_my = nc.dram_tensor("u_my", [B, MS, R], F32, kind="ExternalInput").ap()
    v_full = nc.dram_tensor("v_full", [B, N, R], F32,
                            kind="ExternalInput").ap()
    v_my = nc.dram_tensor("v_my", [B, MS, R], F32, kind="ExternalInput").ap()
    u_out = nc.dram_tensor("u_out", [B, MS, R], F32,
                           kind="ExternalOutput").ap()
    v_out = nc.dram_tensor("v_out", [B, MS, R], F32,
                           kind="ExternalOutput").ap()

    rs_ins = [nc.dram_tensor(f"rs_in_{b}", [NCORES * R, 512 + R], F32)
              for b in range(B)]
    rs_outs = [nc.dram_tensor(f"rs_out_{b}", [R, 512 + R], F32)
               for b in range(B)]

    with tile.TileContext(nc) as tc, ExitStack() as ctx:
        const = ctx.enter_context(tc.tile_pool(name="const", bufs=1))
        xbp = ctx.enter_context(tc.tile_pool(name="xbp", bufs=1))
        xgp = ctx.enter_context(tc.tile_pool(name="xgp", bufs=3))
        xtp = ctx.enter_context(tc.tile_pool(name="xtp", bufs=4))
        vp = ctx.enter_context(tc.tile_pool(name="vp", bufs=2))
        smp = ctx.enter_context(tc.tile_pool(name="smp", bufs=2))
        zsb = ctx.enter_context(tc.tile_pool(name="zsb", bufs=3))
        a2sp = ctx.enter_context(tc.tile_pool(name="a2sp", bufs=3))
        # PSUM: ppt 2 + pa1 1 + pzu 1 + pw 1 + pun 1 + pa2 2 = 8 banks
        ppt = ctx.enter_context(tc.tile_pool(name="ppt", bufs=2,
                                             space="PSUM"))
        pa1p = ctx.enter_context(tc.tile_pool(name="pa1", bufs=1,
                                              space="PSUM"))
        pzup = ctx.enter_context(tc.tile_pool(name="pzu", bufs=1,
                                              space="PSUM"))
        pwp = ctx.enter_context(tc.tile_pool(name="pw", bufs=1,
                                             space="PSUM"))
        punp = ctx.enter_context(tc.tile_pool(name="pun", bufs=1,
                                              space="PSUM"))
        pa2p = ctx.enter_context(tc.tile_pool(name="pa2", bufs=2,
                                              space="PSUM"))

        ident128_b = const.tile([128, 128], BF16)
        make_identity(nc, ident128_b)
        ident128_f = const.tile([128, 128], F32)
        make_identity(nc, ident128_f)
        ident32_f = const.tile([R, R], F32)
        make_identity(nc, ident32_f)
        masksl_f = const.tile([R, R], F32)
        make_lower_triangular(nc, masksl_f, val=1.0, diag=False)
        ident32_r = const.tile([R, R], F32R)
        nc.vector.tensor_copy(ident32_r[:], ident32_f[:])
        masksl_r = const.tile([R, R], F32R)
        nc.vector.tensor_copy(masksl_r[:], masksl_f[:])
        consts = (ident32_r[:], masksl_r[:], ident32_r[:])

        # HAM warm-up: keep PE busy during the first x DMA.
        wz = const.tile([128, 512], BF16)
        nc.gpsimd.memset(wz[:], 0.0)
        for k in range(24):
            pwarm = pa2p.tile([R, MS], F32, tag="pa2", name="pwarm")
            nc.tensor.matmul(pwarm[:], lhsT=wz[:, :R], rhs=wz[:],
                             start=True, stop=True)

        def zps(nm):
            return pzup.tile([R, MS], F32, tag="zu", name=nm)

        def zps_v(nm):
            return pa2p.tile([R, MS], F32, tag="pa2", name=nm)

        state = {}

        def emit_solve_v(b):
            st = state[b]
            a2t = smp.tile([R, MS], F32R, tag="a2t", name="a2t")
            nc.sync.dma_start(a2t[:],
                              rs_outs[b].ap()[:, 0:512].bitcast(F32R))
            b2_sb = smp.tile([R, R], F32R, tag="b2s", name="b2_sb")
            nc.sync.dma_start(b2_sb[:],
                              rs_outs[b].ap()[:, 512:512 + R].bitcast(F32R))
            at2 = smp.tile([R, MS], F32R, tag="at2", name="at2")
            nc.vector.tensor_scalar_add(at2[:], a2t[:], EPS)
            vn32 = smp.tile([128, MC, R], F32, tag="vn32", name="vn32")
            _solve(nc, smp, zsb, pwp, punp, zps_v, consts, b2_sb, at2,
                   st["vT"], vn32, None, "v")
            nc.sync.dma_start(v_out[b].rearrange("(i p) r -> p i r", p=128),
                              vn32[:])

        for b in range(B):
            bi = b % 2
            # ---------------- per-batch loads ----------------
            v32 = vp.tile([128, NCH, R], F32, tag="v32", name="v32")
            nc.sync.dma_start(v32[:],
                              v_full[b].rearrange("(c p) r -> p c r", p=128))
            vb = vp.tile([128, NCH, R], BF16, tag="vb", name="vb")
            nc.vector.tensor_copy(vb[:], v32[:])

            u32 = vp.tile([128, MC, R], F32, tag="u32", name="u32")
            nc.sync.dma_start(u32[:],
                              u_my[b].rearrange("(i p) r -> p i r", p=128))
            vm32 = vp.tile([128, MC, R], F32, tag="vm32", name="vm32")
            nc.sync.dma_start(vm32[:],
                              v_my[b].rearrange("(i p) r -> p i r", p=128))

            # uT / vT via PE transpose
            put = zps("put")
            for i in range(MC):
                nc.tensor.transpose(put[:, i * 128:(i + 1) * 128],
                                    u32[:, i, :], ident128_f[:])
            uT = smp.tile([R, MS], F32R, tag="uT", name="uT")
            nc.scalar.copy(uT[:], put[:])
            pvt = zps("pvt")
            for i in range(MC):
                nc.tensor.transpose(pvt[:, i * 128:(i + 1) * 128],
                                    vm32[:, i, :], ident128_f[:])
            vT = smp.tile([R, MS], F32R, tag="vT", name="vT")
            nc.scalar.copy(vT[:], pvt[:])

            # ---------------- phase 1: stream x ----------------
            xb_t = xbp.tile([128, MC, N], BF16, tag=f"xb{bi}", name="xb")
            pa1 = pa1p.tile([R, MS], F32, tag="pa1", name="pa1")
            pb1 = zps("pb1")  # [R, MS] slot; use [:, :R] for the Gram
            x_re = x_my[b].rearrange("(i p) n -> p i n", p=128)
            for g in range(NG):
                xg = xgp.tile([128, MC, 512], F32, tag="xg", name="xg")
                nc.sync.dma_start(xg[:], x_re[:, :, g * 512:(g + 1) * 512])
                nc.vector.tensor_copy(xb_t[:, :, g * 512:(g + 1) * 512],
                                      xg[:])
                for j2 in range(4):
                    j = 4 * g + j2
                    pt = ppt.tile([128, MC, 128], BF16, tag="pt", name="pt")
                    for i in range(MC):
                        nc.tensor.transpose(
                            pt[:, i], xb_t[:, i, j * 128:(j + 1) * 128],
                            ident128_b[:])
                    xt = xtp.tile([128, MC, 128], BF16, tag="xt", name="xt")
                    nc.scalar.copy(xt[:], pt[:])
                    nc.tensor.matmul(pa1[:], lhsT=vb[:, j, :],
                                     rhs=xt.rearrange("p a b -> p (a b)"),
                                     start=(j == 0), stop=(j == NCH - 1),
                                     skip_group_check=True)
                    nc.tensor.matmul(pb1[:, :R], lhsT=vb[:, j, :],
                                     rhs=vb[:, j, :], start=(j == 0),
                                     stop=(j == NCH - 1),
                                     skip_group_check=True)

            # ---------------- u solve ----------------
            b1_sb = smp.tile([R, R], F32R, tag="b1s", name="b1_sb")
            nc.scalar.copy(b1_sb[:], pb1[:, :R])
            at1 = smp.tile([R, MS], F32R, tag="at1", name="at1")
            nc.vector.tensor_scalar_add(at1[:], pa1[:], EPS)
            un32 = smp.tile([128, MC, R], F32, tag="un32", name="un32")
            unb = smp.tile([128, MC, R], BF16, tag="unb", name="unb")
            _solve(nc, smp, zsb, pwp, punp, zps, consts, b1_sb, at1, uT,
                   un32, unb, "u")
            nc.sync.dma_start(u_out[b].rearrange("(i p) r -> p i r", p=128),
                              un32[:])

            # ---------------- phase 2 partials + RS ----------------
            for g2 in range(NG):
                pa2 = pa2p.tile([R, MS], F32, tag="pa2", name="pa2")
                for i in range(MC):
                    nc.tensor.matmul(
                        pa2[:], lhsT=unb[:, i, :],
                        rhs=xb_t[:, i, g2 * 512:(g2 + 1) * 512],
                        start=(i == 0), stop=(i == MC - 1),
                        skip_group_check=True)
                a2st = a2sp.tile([R, MS], F32, tag="a2st", name="a2st")
                nc.scalar.copy(a2st[:], pa2[:])
                nc.sync.dma_start(rs_ins[b].ap()[ds(g2 * R, R), 0:512],
                                  a2st[:])
            pb2 = pwp.tile([R, R], F32, tag="pw", name="pb2")
            for i in range(MC):
                nc.tensor.matmul(pb2[:], lhsT=unb[:, i, :], rhs=unb[:, i, :],
                                 start=(i == 0), stop=(i == MC - 1),
                                 skip_group_check=True)
            b2st = a2sp.tile([R, R], F32, tag="b2st", name="b2st")
            nc.scalar.copy(b2st[:], pb2[:])
            for c in range(NCORES):
                nc.sync.dma_start(
                    rs_ins[b].ap()[ds(c * R, R), 512:512 + R], b2st[:])

            nc.gpsimd.collective_compute(
                "ReduceScatter", ALU.add,
                replica_groups=[list(range(NCORES))],
                ins=[rs_ins[b].ap()], outs=[rs_outs[b].ap()])
            state[b] = {"vT": vT}

            # v-solve of the previous batch (its RS has completed by now)
            if b >= 1:
                emit_solve_v(b - 1)

        emit_solve_v(B - 1)

    nc.compile()
    return nc


def kernel(x, u, v):
    global LAST_RESULT
    if "nc" not in _CACHE:
        _CACHE["nc"] = _build()
    nc = _CACHE["nc"]

    x = np.ascontiguousarray(x, dtype=np.float32)
    u = np.ascontiguousarray(u, dtype=np.float32)
    v = np.ascontiguousarray(v, dtype=np.float32)

    in_maps = []
    for c in range(NCORES):
        sl = slice(c * MS, (c + 1) * MS)
        in_maps.append({
            "x_my": np.ascontiguousarray(x[:, sl, :]),
            "u_my": np.ascontiguousarray(u[:, sl, :]),
            "v_full": v,
            "v_my": np.ascontiguousarray(v[:, sl, :]),
        })

    res = run_bass_kernel_spmd(nc, in_maps, list(range(NCORES)),
                               trace=os.environ.get("KBENCH_TRACE") == "1")
    LAST_RESULT = res
    u_new = np.concatenate([res.results[c]["u_out"] for c in range(NCORES)],
                           axis=1)
    v_new = np.concatenate([res.results[c]["v_out"] for c in range(NCORES)],
                           axis=1)
    return (u_new, v_new)
